# revision 38
# baseline (speedup 1.0000x reference)
"""3-layer GCN (gcn_norm + 3x gcn_conv + log_softmax) on 8 TRN2 NeuronCores.

v3 strategy (dst-sharded, graph-parallel), changes vs v2:
  - scatter_add + accumulator-zeroing + readback (~320us of DMA-queue time)
    replaced by: fp16 token-partial DRAM write (linear, 256B-stride rows) +
    per-group canonical dma_gather back into SBUF + DVE accumulate. No RMW,
    no zero pass, 1x readback instead of 4x, half the permute bytes.
  - copy-first plane reduction: plane 0 of every group covers all active
    blocks densely (padding slots point at a guaranteed-zero table row), so
    the first plane is a copy, not add-into-memset. All aggw memsets gone;
    single aggw buffer (the fp16 cast frees it immediately).
  - host-expanded dinvx64/dinvx16/b3full tiles replaced by stride-0
    broadcast APs over small [P, NB] / [P, 16] tiles (saves ~4.8MB of
    per-call loads and the same SBUF).
  - deeper slot/cg pools (8/3): DMA pipelining worth ~0.9ms.
  - SWDGE calls stay at num_idxs <= 1024: the descriptor ring (16KB
    scratch / 16B per desc) is a hard per-call limit; 2048-idx calls hang
    the device even with a larger walrus scratch flag.
  - GCN_REPS env loops the whole body inside one NEFF for differential
    timing (per-body ns = (T(K)-T(1))/(K-1), immune to dispatch floor).

kernel(**inputs) takes FULL unsharded inputs, returns FULL [100000, 10]
float32 log-softmax output.
"""
import dataclasses
import os
import sys

import numpy as np

if "/opt/trn_rl_repo" not in sys.path:
    sys.path.insert(0, "/opt/trn_rl_repo")

N_NODES = 100000
N_EDGES = 1600000
N_CORES = 8
P = 128
NB = 98                      # node blocks per core
NLOC = P * NB                # 12544 padded nodes per core
NPC = N_NODES // N_CORES     # 12500 real nodes per core
NPAD = N_CORES * NLOC        # 100352 table rows
NG = 4                       # gather groups (source-row parity)
NJ = NPAD // NG              # stride positions per group (25088 < 32768)
GCHUNK = int(os.environ.get("GCN_GCHUNK", "8"))   # slot cols per gather call
CCH = int(os.environ.get("GCN_CCH", "8"))   # canonical blocks per call
DMA_SCRATCH = int(os.environ.get("GCN_SCRATCH", "16384"))


# --------------------------------------------------------------------------
# host-side graph preprocessing (integer/layout work only)
# --------------------------------------------------------------------------

def _prep(edge_index):
    src = np.asarray(edge_index[0], dtype=np.int64)
    dst = np.asarray(edge_index[1], dtype=np.int64)

    deg_in = np.bincount(dst, minlength=N_NODES)
    core_of = np.arange(N_NODES) // NPC

    # canonical within-core order: total degree descending. Dummy (pad)
    # ranks are the tail plus {12414, 12415} (= positions (126,96),(127,96)),
    # chosen so dummy rows cover all 4 source-row parities (the natural tail
    # column 97 only yields odd parities; zero-gather targets need all 4).
    dummy_ranks = np.array([12414, 12415] + list(range(NPC + 2, NLOC)),
                           dtype=np.int64)
    real_ranks = np.setdiff1d(np.arange(NLOC), dummy_ranks)
    assert len(real_ranks) == NPC
    rank = np.empty(N_NODES, dtype=np.int64)
    for c in range(N_CORES):
        ids = np.arange(c * NPC, (c + 1) * NPC)
        o = np.argsort(-deg_in[ids], kind="stable")
        rank[ids[o]] = real_ranks
    p_of = rank % P
    b_of = rank // P
    r_loc = p_of * NB + b_of                 # canonical local row
    g_row = core_of * NLOC + r_loc           # global table row

    # message entries = edges only (self loops are handled locally on-core
    # by adding each core's own table rows to its aggregate)
    all_dst = dst
    all_gsrc = g_row[src]
    all_q = all_gsrc % NG
    all_j = all_gsrc // NG
    all_core = all_dst // NPC

    # per (node, group) counts
    qcnt = np.bincount(all_dst * NG + all_q,
                       minlength=N_NODES * NG).reshape(N_NODES, NG)

    # per (core, group): group-degree-sorted token ranks
    qrank = np.empty((N_NODES, NG), dtype=np.int64)
    qdeg_sorted = np.zeros((N_CORES, NG, NLOC), dtype=np.int64)
    for c in range(N_CORES):
        ids = np.arange(c * NPC, (c + 1) * NPC)
        for q in range(NG):
            o = np.argsort(-qcnt[ids, q], kind="stable")
            qrank[ids[o], q] = np.arange(NPC)
            qdeg_sorted[c, q, :NPC] = qcnt[ids[o], q]

    # shared slot structure (cross-core max; non-increasing in b)
    S = [qdeg_sorted[:, q, ::P].max(axis=0).astype(np.int64)
         for q in range(NG)]
    B = []
    for q in range(NG):
        s0 = int(S[q][0])
        B.append([int((S[q] > k).sum()) for k in range(s0)])
    SW = [int(S[q].sum()) for q in range(NG)]
    plane_off = [np.concatenate([[0], np.cumsum(B[q])]).astype(np.int64)
                 for q in range(NG)]
    B0 = [B[q][0] for q in range(NG)]

    # guaranteed-zero gather target per group (a dummy node row)
    dummy_rloc = (dummy_ranks % P) * NB + dummy_ranks // P
    all_dummies = (np.arange(N_CORES)[:, None] * NLOC +
                   dummy_rloc[None, :]).ravel()
    zero_j = []
    for q in range(NG):
        cand = all_dummies[all_dummies % NG == q]
        assert len(cand) > 0, f"no zero row in group {q}"
        zero_j.append(int(cand[0] // NG))

    # fill gather index arrays (slot col of k-th entry = plane_off[k] + b)
    idx_lin = [np.full((N_CORES, SW[q] * P), zero_j[q], dtype=np.int64)
               for q in range(NG)]
    ekey = (all_core * NG + all_q) * NLOC + qrank[all_dst, all_q]
    order = np.argsort(ekey, kind="stable")
    se_key = ekey[order]
    se_j = all_j[order]
    is_start = np.ones(len(se_key), dtype=bool)
    is_start[1:] = se_key[1:] != se_key[:-1]
    grp_start_pos = np.flatnonzero(is_start)
    grp_id = np.cumsum(is_start) - 1
    k_within = np.arange(len(se_key)) - grp_start_pos[grp_id]

    se_c = se_key // (NG * NLOC)
    se_q = (se_key // NLOC) % NG
    se_r = se_key % NLOC
    se_p = se_r % P
    se_b = se_r // P
    for q in range(NG):
        m = se_q == q
        col = plane_off[q][k_within[m]] + se_b[m]
        idx_lin[q][se_c[m], col * P + se_p[m]] = se_j[m]
        assert idx_lin[q].min() >= 0 and idx_lin[q].max() < NJ

    # canonical-gather indices: canonical row (p, b) of this core reads
    # token t = qrank[node, q] at token-array row (t % P) * B0q + t // P.
    # Tokens at rank >= B0q*P (zero-degree tail + dummies) are never
    # written; remap them to a written zero-degree token of this (c, q).
    cidx_lin = [np.zeros((N_CORES, NLOC), dtype=np.int64) for q in range(NG)]
    for c in range(N_CORES):
        ids = np.arange(c * NPC, (c + 1) * NPC)
        for q in range(NG):
            lim = B0[q] * P
            zpos = np.flatnonzero(qdeg_sorted[c, q, :lim] == 0)
            assert len(zpos) > 0, f"no zero token in range for c{c} q{q}"
            zt = int(zpos[0])
            tok = np.full(NLOC, zt, dtype=np.int64)       # dummies -> zt
            tq = qrank[ids, q]
            tq = np.where(tq < lim, tq, zt)               # tail -> zt
            tok[r_loc[ids]] = tq
            # linear layout: index for out (p, col b) at position b*P + p,
            # where canonical row r = p*NB + b
            j = (tok % P) * B0[q] + tok // P
            lin = np.empty(NLOC, dtype=np.int64)
            lin[(np.arange(NLOC) % NB) * P + np.arange(NLOC) // NB] = j
            cidx_lin[q][c] = lin
            assert lin.min() >= 0 and lin.max() < B0[q] * P

    IDXREP = int(os.environ.get("GCN_IDXREP", "8"))

    def wrap16(lin2d):
        # [cores, L] -> [cores, 16*IDXREP, L//16] with linear[i] at
        # [:, i%16, i//16] (ucode reads indices from 16 partitions;
        # IDXREP=8 replicates to 128 partitions if the ucode needs it).
        n = lin2d.shape[1]
        a = lin2d.reshape(N_CORES, n // 16, 16).transpose(0, 2, 1)
        a = np.ascontiguousarray(a).astype(np.int16)
        return np.tile(a, (1, IDXREP, 1))

    idx_cat = np.concatenate([wrap16(idx_lin[q]) for q in range(NG)], axis=2)
    idx_qoff = np.concatenate(
        [[0], np.cumsum([SW[q] * 8 for q in range(NG)])]).astype(int)

    cidx_cat = np.concatenate([wrap16(cidx_lin[q]) for q in range(NG)],
                              axis=2)
    cidx_qoff = np.concatenate(
        [[0], np.cumsum([NLOC // 16 for q in range(NG)])]).astype(int)

    meta = dict(S=[s.tolist() for s in S], B=B, SW=SW, B0=B0,
                plane_off=[p.tolist() for p in plane_off],
                idx_qoff=idx_qoff.tolist(), cidx_qoff=cidx_qoff.tolist())
    perm = dict(core_of=core_of, r_loc=r_loc, deg_in=deg_in,
                p_of=p_of, b_of=b_of, g_row=g_row)
    return meta, perm, idx_cat, cidx_cat


def _pack_inputs(x, W1, b1, W2, b2, W3, b3, perm, idx_cat, cidx_cat):
    core_of, r_loc = perm["core_of"], perm["r_loc"]
    p_of, b_of, deg_in = perm["p_of"], perm["b_of"], perm["deg_in"]
    g_row = perm["g_row"]

    dinv_host = 1.0 / np.sqrt((deg_in + 1).astype(np.float64))

    # conv1 source table: x * dinv in canonical global layout (zeros at pads)
    xt = np.zeros((NPAD, 16), dtype=np.float32)
    xt[g_row, :13] = x * dinv_host[:, None].astype(np.float32)

    # per-core dinv tile [P, NB] (0 at dummy rows); broadcast on-device
    dinv_pb = np.zeros((N_CORES, P, NB), dtype=np.float32)
    dinv_pb[core_of, p_of, b_of] = dinv_host.astype(np.float32)

    # bias tiles
    bt1x = np.tile(b1, 4)[None, :].repeat(P, 0).astype(np.float32)  # [P,256]
    bt2x = np.tile(b2, 2)[None, :].repeat(P, 0).astype(np.float32)  # [P,128]
    b3blk = np.full(16, -1e30, np.float32)
    b3blk[:10] = b3
    b3t = b3blk[None, :].repeat(P, 0).astype(np.float32)            # [P,16]

    # block-diagonal weights
    W1p = np.zeros((16, 64), np.float32)
    W1p[:13] = W1
    rhs1 = np.zeros((64, 256), np.float32)
    for g in range(4):
        rhs1[g * 16:(g + 1) * 16, g * 64:(g + 1) * 64] = W1p
    rhs2 = np.zeros((128, 128), np.float32)
    rhs2[:64, :64] = W2
    rhs2[64:, 64:] = W2
    W3p = np.zeros((64, 16), np.float32)
    W3p[:, :10] = W3
    rhs3 = np.zeros((128, 32), np.float32)
    rhs3[:64, :16] = W3p
    rhs3[64:, 16:] = W3p

    # per-core own slice of xt in canonical [P, NB*16] layout (self loops)
    xt_own = np.ascontiguousarray(
        xt.reshape(N_CORES, P, NB, 16)).reshape(N_CORES, P, NB * 16)

    return [{
        "gidx": idx_cat[c], "cidx": cidx_cat[c],
        "xt": xt, "xt_own": xt_own[c],
        "dinv_pb": dinv_pb[c].reshape(P, NB),
        "bt1x": bt1x, "bt2x": bt2x, "b3t": b3t,
        "rhs1": rhs1, "rhs2": rhs2, "rhs3": rhs3,
    } for c in range(N_CORES)]


# --------------------------------------------------------------------------
# raw dma_gather emitter: bass.BassGpSimd.dma_gather minus the elem%256B
# restriction (the q7 ucode only requires the row *stride* to be a multiple
# of 256B for the non-transpose HBM path).
# --------------------------------------------------------------------------

def _dma_gather_raw(eng, out_ap, in_ap, idxs_ap, num_idxs, elem_size,
                    elem_step, queue_num=0):
    import concourse.mybir as mybir
    from concourse import ap_utils
    from concourse.bass import MemorySpace

    assert idxs_ap.dtype == mybir.dt.int16
    assert in_ap.dtype == out_ap.dtype
    assert in_ap.space == MemorySpace.DRAM
    assert idxs_ap.space == MemorySpace.SBUF
    assert out_ap.space == MemorySpace.SBUF
    assert ap_utils.ap_is_contiguous(out_ap.ap[1:])
    assert ap_utils.ap_is_contiguous(idxs_ap.ap[1:])
    assert num_idxs % P == 0
    assert out_ap.ap[0][1] * out_ap.ap[1][1] == num_idxs
    assert in_ap.ap[-1][1] == out_ap.ap[-1][1] == elem_size
    assert in_ap.ap[0][0] == elem_step
    stride_bytes = elem_step * mybir.dt.size(in_ap.dtype)
    assert stride_bytes % 256 == 0
    stride_bytes_256 = stride_bytes // 256
    assert 0 < stride_bytes_256 < 256

    _in_ap = eng.lower_ap_dma(in_ap, for_custom_bir_dma=True)
    _idxs_ap = eng.lower_ap(idxs_ap)
    _out_ap = eng.lower_ap(out_ap)
    return eng.add_instruction(
        mybir.InstDMAGatherAnt(
            name=eng.bass.get_next_instruction_name(),
            ins=[*_in_ap, _idxs_ap,
                 eng.lower_val_access(eng.to_reg(num_idxs))],
            outs=[_out_ap],
            transpose=False,
            num_idxs=num_idxs,
            elem_size=elem_size,
            stride_bytes_256=stride_bytes_256,
            gen_mode=0,
            single_packet=os.environ.get("GCN_SP", "1") == "1",
            queue_num=queue_num,
            sbuf_tokens_per_rank=0,
            sbuf_free_dim_per_rank=0,
            sbuf_free_dim_pad_per_rank=0,
            sbuf_byte_offset=0,
        ))


# --------------------------------------------------------------------------
# device program
# --------------------------------------------------------------------------

def _patch_walrus_scratch(size):
    """Pass the SWDGE descriptor-ring size to walrus (the BIR-lowering
    payload doesn't carry dynamic_dma_scratch_size; walrus defaults to
    16KB/partition = 1024 descs/queue). Must match Bacc's param so bass
    reserves the same SBUF top region walrus will use."""
    from concourse import bass_utils as bu
    if getattr(bu, "_scratch_patch_size", None) == size:
        return
    orig = getattr(bu, "_orig_get_walrus_args", None) or bu.get_walrus_args
    bu._orig_get_walrus_args = orig

    def patched(*a, **k):
        return [f"--dynamic-dma-scratch-size-per-partition={size}",
                *orig(*a, **k)]

    bu.get_walrus_args = patched
    bu._scratch_patch_size = size


def _build(meta):
    import concourse.bacc as bacc
    import concourse.mybir as mybir
    import concourse.tile as tile
    from concourse.masks import make_identity

    if DMA_SCRATCH != 16384:
        _patch_walrus_scratch(DMA_SCRATCH)

    f32 = mybir.dt.float32
    f16 = mybir.dt.float16
    AF = mybir.ActivationFunctionType
    OP = mybir.AluOpType

    S, B, SW, B0 = meta["S"], meta["B"], meta["SW"], meta["B0"]
    plane_off = meta["plane_off"]
    idx_qoff, cidx_qoff = meta["idx_qoff"], meta["cidx_qoff"]
    B0M = max(B0)

    REPS = int(os.environ.get("GCN_REPS", "1"))

    IDXP = 16 * int(os.environ.get("GCN_IDXREP", "8"))

    nc = bacc.Bacc(num_swdge_queues=4,
                   dynamic_dma_scratch_size=DMA_SCRATCH)
    gidx_in = nc.declare_dram_parameter("gidx", [IDXP, idx_qoff[-1]],
                                        mybir.dt.int16, isOutput=False)
    cidx_in = nc.declare_dram_parameter("cidx", [IDXP, cidx_qoff[-1]],
                                        mybir.dt.int16, isOutput=False)
    xt_in = nc.declare_dram_parameter("xt", [NPAD, 16], f32, isOutput=False)
    xt_own_in = nc.declare_dram_parameter("xt_own", [P, NB * 16], f32,
                                          isOutput=False)
    dinv_in = nc.declare_dram_parameter("dinv_pb", [P, NB], f32,
                                        isOutput=False)
    bt1x_in = nc.declare_dram_parameter("bt1x", [P, 256], f32, isOutput=False)
    bt2x_in = nc.declare_dram_parameter("bt2x", [P, 128], f32,
                                        isOutput=False)
    b3t_in = nc.declare_dram_parameter("b3t", [P, 16], f32, isOutput=False)
    rhs1_in = nc.declare_dram_parameter("rhs1", [64, 256], f32,
                                        isOutput=False)
    rhs2_in = nc.declare_dram_parameter("rhs2", [128, 128], f32,
                                        isOutput=False)
    rhs3_in = nc.declare_dram_parameter("rhs3", [128, 32], f32,
                                        isOutput=False)
    out_ext = nc.declare_dram_parameter("out", [NLOC, 16], f32, isOutput=True)

    agin2 = nc.dram_tensor("agin2", [NLOC, 64], f16)
    table2 = nc.dram_tensor("table2", [NPAD, 64], f16, addr_space="Shared")
    agin3 = nc.dram_tensor("agin3", [NLOC, 16], f32)
    table3 = nc.dram_tensor("table3", [NPAD, 16], f32, addr_space="Shared")
    # token-partial arrays, one per (conv, group): fp16 rows, stride 256B
    # (128 f16); conv1/conv3 use the first 16 columns, conv2 the first 64
    tokds = [[nc.dram_tensor(f"tokd{i}_{q}", [P * B0[q], 128], f16)
              for q in range(NG)] for i in range(3)]

    rg = [list(range(N_CORES))]

    with tile.TileContext(nc) as tc:
        with (
            tc.tile_pool(name="persist", bufs=1) as pp,
            tc.tile_pool(name="slots", bufs=int(os.environ.get("GCN_SBUFS", "12"))) as sp,
            tc.tile_pool(name="cg", bufs=int(os.environ.get("GCN_CGBUFS", "3"))) as cgp,
            tc.tile_pool(name="tmp", bufs=int(os.environ.get("GCN_TBUFS", "8"))) as tp,
            tc.tile_pool(name="psum", bufs=int(os.environ.get("GCN_PBUFS", "4")), space="PSUM") as psp,
        ):
            # ---- persistent inputs ----
            gidx_t = pp.tile([IDXP, idx_qoff[-1]], mybir.dt.int16)
            for q in range(NG):  # per-group loads: group 0 gathers start
                nc.sync.dma_start(  # before the whole table is resident
                    out=gidx_t[:, idx_qoff[q]:idx_qoff[q + 1]],
                    in_=gidx_in[:, idx_qoff[q]:idx_qoff[q + 1]])
            cidx_t = pp.tile([IDXP, cidx_qoff[-1]], mybir.dt.int16)
            nc.sync.dma_start(out=cidx_t[:], in_=cidx_in[:, :])
            dinv_t = pp.tile([P, NB], f32)
            nc.sync.dma_start(out=dinv_t[:], in_=dinv_in[:, :])
            bt1x_t = pp.tile([P, 256], f32)
            nc.sync.dma_start(out=bt1x_t[:], in_=bt1x_in[:, :])
            bt2x_t = pp.tile([P, 128], f32)
            nc.sync.dma_start(out=bt2x_t[:], in_=bt2x_in[:, :])
            b3t_t = pp.tile([P, 16], f32)
            nc.sync.dma_start(out=b3t_t[:], in_=b3t_in[:, :])
            rhs1_t = pp.tile([64, 256], f32)
            nc.sync.dma_start(out=rhs1_t[:], in_=rhs1_in[:, :])
            rhs2_t = pp.tile([128, 128], f32)
            nc.sync.dma_start(out=rhs2_t[:], in_=rhs2_in[:, :])
            rhs3_t = pp.tile([128, 32], f32)
            nc.sync.dma_start(out=rhs3_t[:], in_=rhs3_in[:, :])
            ident = pp.tile([P, P], f32)
            make_identity(nc, ident[:])

            aggw = pp.tile([P, B0M * 64], f32, name="aggw")
            cast16 = pp.tile([P, B0M * 64], f16, name="cast16")
            out_w = pp.tile([P, NB * 64], f32)
            x1 = pp.tile([P, NB * 64], f32)
            shard16 = pp.tile([P, NB * 64], f16)
            shard3 = pp.tile([P, NB * 16], f32)
            mx = pp.tile([P, NB], f32)
            sm = pp.tile([P, NB], f32)
            lg = pp.tile([P, NB], f32)
            qctr = [0]

            phase = os.environ.get("GCN_PHASE", "")
            chunkadd = os.environ.get("GCN_CHUNKADD", "0") == "1"

            # broadcast-AP helpers: value tile [P, NB] (or [P, 16]) read with
            # a stride-0 innermost axis so one scalar covers F columns
            def _bcast_blk(t, g0, gsz, F):
                # [P, gsz, F] view of t[:, g0:g0+gsz] broadcast over F
                ap = t[:, g0:g0 + gsz]
                return dataclasses.replace(ap, ap=type(ap.ap)(
                    [list(ap.ap[0]), [1, gsz], [0, F]]))

            def _bcast_row16(t, nb_):
                # [P, nb_, 16] view of a [P, 16] tile broadcast over blocks
                ap = t[:, :]
                return dataclasses.replace(ap, ap=type(ap.ap)(
                    [list(ap.ap[0]), [0, nb_], [1, 16]]))

            def aggregate(base_view, elem, dt, fl, tokd4, self_tile):
                # out_w holds the running aggregate in [P, NB, fl] layout
                if self_tile is None:
                    nc.sync.dma_start(
                        out=out_w[:, :NB * fl].rearrange(
                            "p (b f) -> p b f", b=NB),
                        in_=xt_own_in[:, :].rearrange(
                            "p (b f) -> p b f", b=NB))
                else:
                    nc.vector.tensor_scalar(out=out_w[:, :NB * fl],
                                            in0=self_tile[:], scalar1=0.0,
                                            scalar2=None, op0=OP.add)
                if phase == "noagg":
                    return out_w
                for q in range(NG):
                    in_view = base_view[:, q * elem:(q + 1) * elem]
                    c0 = 0
                    while c0 < SW[q]:
                        c1 = min(c0 + GCHUNK, SW[q])
                        ncols = c1 - c0
                        st = sp.tile([P, GCHUNK * elem], dt, tag="slot")
                        _dma_gather_raw(
                            nc.gpsimd,
                            out_ap=st[:, :ncols * elem].rearrange(
                                "p (c f) -> p c f", c=ncols),
                            in_ap=in_view,
                            idxs_ap=gidx_t[:, idx_qoff[q] + c0 * 8:
                                           idx_qoff[q] + c1 * 8],
                            num_idxs=ncols * P,
                            elem_size=elem, elem_step=base_view.ap[0][0],
                            queue_num=qctr[0] % 4)
                        qctr[0] += 1
                        for k in range(len(B[q])):
                            s0 = max(c0, int(plane_off[q][k]))
                            s1 = min(c1, int(plane_off[q][k + 1]))
                            if s0 >= s1:
                                continue
                            bs = s0 - int(plane_off[q][k])
                            be = s1 - int(plane_off[q][k])
                            if k == 0:
                                # first touch of these blocks: copy
                                nc.vector.tensor_scalar(
                                    out=aggw[:, bs * fl:be * fl],
                                    in0=st[:, (s0 - c0) * elem:
                                           (s1 - c0) * elem],
                                    scalar1=0.0, scalar2=None, op0=OP.add)
                            else:
                                nc.vector.tensor_tensor(
                                    out=aggw[:, bs * fl:be * fl],
                                    in0=aggw[:, bs * fl:be * fl],
                                    in1=st[:, (s0 - c0) * elem:
                                           (s1 - c0) * elem],
                                    op=OP.add)
                        c0 = c1
                    if phase == "noperm":
                        continue
                    # fp16 cast frees aggw immediately; token write + gather
                    # move half the bytes
                    nc.vector.tensor_scalar(
                        out=cast16[:, :B0[q] * fl],
                        in0=aggw[:, :B0[q] * fl],
                        scalar1=0.0, scalar2=None, op0=OP.add)
                    nc.sync.dma_start(
                        out=tokd4[q][:, 0:fl].rearrange(
                            "(p b) f -> p b f", p=P),
                        in_=cast16[:, :B0[q] * fl].rearrange(
                            "p (b f) -> p b f", b=B0[q]))
                    # canonical gather + accumulate (ring-sized chunks);
                    # per-chunk adds let downstream matmuls start on early
                    # blocks while later chunks are still gathering
                    cg = cgp.tile([P, NB * 64], f16, tag="cg")
                    for cb0 in range(0, NB, CCH):
                        nb_ = min(CCH, NB - cb0)
                        _dma_gather_raw(
                            nc.gpsimd,
                            out_ap=cg[:, cb0 * fl:(cb0 + nb_) * fl].rearrange(
                                "p (c f) -> p c f", c=nb_),
                            in_ap=tokd4[q][:, 0:fl],
                            idxs_ap=cidx_t[:, cidx_qoff[q] + cb0 * 8:
                                           cidx_qoff[q] + (cb0 + nb_) * 8],
                            num_idxs=nb_ * P,
                            elem_size=fl, elem_step=128,
                            queue_num=qctr[0] % 4)
                        qctr[0] += 1
                        if chunkadd:
                            nc.vector.tensor_tensor(
                                out=out_w[:, cb0 * fl:(cb0 + nb_) * fl],
                                in0=out_w[:, cb0 * fl:(cb0 + nb_) * fl],
                                in1=cg[:, cb0 * fl:(cb0 + nb_) * fl],
                                op=OP.add)
                    if not chunkadd:
                        nc.vector.tensor_tensor(
                            out=out_w[:, :NB * fl],
                            in0=out_w[:, :NB * fl],
                            in1=cg[:, :NB * fl], op=OP.add)
                return out_w

            xt_view = xt_in[:, :].rearrange("(j t) f -> j (t f)", t=NG)
            t2_view = table2[:, :].rearrange("(j t) f -> j (t f)", t=NG)
            t3_view = table3[:, :].rearrange("(j t) f -> j (t f)", t=NG)

            for rep in range(REPS):
                # =============== conv1 ===============
                agg1 = aggregate(xt_view, 16, f32, 16, tokds[0], None)

                if phase == "nomm":  # timing probe: skip matmul chains
                    nc.vector.memset(x1[:], 0.01)
                    nc.vector.memset(shard16[:], 0.01)
                    nc.vector.memset(shard3[:], 0.01)

                # conv1 compute: x1 = relu((agg1*dinv) @ W1 + b1)
                g0 = 0
                while g0 < NB and phase != "nomm":
                    gsz = min(4, NB - g0)
                    cols = slice(g0 * 16, (g0 + gsz) * 16)
                    t1 = tp.tile([P, 64], f32, tag="t1")
                    nc.vector.tensor_tensor(
                        out=t1[:, :gsz * 16].rearrange(
                            "p (b f) -> p b f", b=gsz),
                        in0=agg1[:, cols].rearrange(
                            "p (b f) -> p b f", b=gsz),
                        in1=_bcast_blk(dinv_t, g0, gsz, 16), op=OP.mult)
                    tps = psp.tile([64, P], f32, tag="tp")
                    nc.tensor.transpose(out=tps[:gsz * 16, :],
                                        in_=t1[:, :gsz * 16],
                                        identity=ident[:])
                    lhsT = tp.tile([64, P], f32, tag="lhsT")
                    nc.scalar.copy(lhsT[:gsz * 16, :], tps[:gsz * 16, :])
                    mm = psp.tile([P, 256], f32, tag="mm")
                    nc.tensor.matmul(out=mm[:, :gsz * 64],
                                     lhsT=lhsT[:gsz * 16, :],
                                     rhs=rhs1_t[:gsz * 16, :gsz * 64],
                                     start=True, stop=True)
                    ocols = slice(g0 * 64, (g0 + gsz) * 64)
                    nc.vector.tensor_tensor(out=x1[:, ocols],
                                            in0=mm[:, :gsz * 64],
                                            in1=bt1x_t[:, :gsz * 64],
                                            op=OP.add)
                    nc.vector.tensor_scalar(out=x1[:, ocols],
                                            in0=x1[:, ocols],
                                            scalar1=0.0, scalar2=None,
                                            op0=OP.max)
                    g0 += gsz

                # =============== conv2 table: (x1 @ W2) * dinv -> fp16
                for g in range(0 if phase == "nomm" else NB // 2):
                    cols = slice(g * 128, (g + 1) * 128)
                    tps = psp.tile([128, P], f32, tag="tp")
                    nc.tensor.transpose(out=tps[:], in_=x1[:, cols],
                                        identity=ident[:])
                    lhsT = tp.tile([128, P], f32, tag="lhsT")
                    nc.scalar.copy(lhsT[:], tps[:])
                    mm = psp.tile([P, 256], f32, tag="mm")
                    nc.tensor.matmul(out=mm[:, :128], lhsT=lhsT[:],
                                     rhs=rhs2_t[:, :], start=True, stop=True)
                    nc.vector.tensor_tensor(
                        out=shard16[:, cols].rearrange(
                            "p (b f) -> p b f", b=2),
                        in0=mm[:, :128].rearrange("p (b f) -> p b f", b=2),
                        in1=_bcast_blk(dinv_t, 2 * g, 2, 64), op=OP.mult)
                nc.sync.dma_start(
                    out=agin2[:, :].rearrange("(p b) f -> p b f", p=P),
                    in_=shard16[:].rearrange("p (b f) -> p b f", b=NB))
                if phase != "nocoll":
                    nc.gpsimd.collective_compute(
                        "AllGather", OP.bypass, replica_groups=rg,
                        ins=[agin2[:, :]], outs=[table2[:, :]])

                agg2 = aggregate(t2_view, 64, f16, 64, tokds[1], shard16)

                # conv2 epilogue: x2 = relu(agg2*dinv + b2) + x1  (into x1)
                nc.vector.tensor_tensor(
                    out=agg2[:].rearrange("p (b f) -> p b f", b=NB),
                    in0=agg2[:].rearrange("p (b f) -> p b f", b=NB),
                    in1=_bcast_blk(dinv_t, 0, NB, 64), op=OP.mult)
                for g in range(NB // 2):
                    cols = slice(g * 128, (g + 1) * 128)
                    nc.vector.tensor_tensor(out=agg2[:, cols],
                                            in0=agg2[:, cols],
                                            in1=bt2x_t[:], op=OP.add)
                nc.vector.tensor_scalar(out=agg2[:], in0=agg2[:],
                                        scalar1=0.0, scalar2=None, op0=OP.max)
                nc.vector.tensor_tensor(out=x1[:], in0=x1[:], in1=agg2[:],
                                        op=OP.add)

                # =============== conv3 table: (x2 @ W3) * dinv -> fp16 [,16]
                for g in range(0 if phase == "nomm" else NB // 2):
                    cols = slice(g * 128, (g + 1) * 128)
                    tps = psp.tile([128, P], f32, tag="tp")
                    nc.tensor.transpose(out=tps[:], in_=x1[:, cols],
                                        identity=ident[:])
                    lhsT = tp.tile([128, P], f32, tag="lhsT")
                    nc.scalar.copy(lhsT[:], tps[:])
                    mm = psp.tile([P, 256], f32, tag="mm")
                    nc.tensor.matmul(out=mm[:, :32], lhsT=lhsT[:],
                                     rhs=rhs3_t[:, :], start=True, stop=True)
                    ocols = slice(g * 32, (g + 1) * 32)
                    nc.vector.tensor_tensor(
                        out=shard3[:, ocols].rearrange(
                            "p (b f) -> p b f", b=2),
                        in0=mm[:, :32].rearrange("p (b f) -> p b f", b=2),
                        in1=_bcast_blk(dinv_t, 2 * g, 2, 16), op=OP.mult)
                nc.sync.dma_start(
                    out=agin3[:, :].rearrange("(p b) f -> p b f", p=P),
                    in_=shard3[:].rearrange("p (b f) -> p b f", b=NB))
                if phase != "nocoll":
                    nc.gpsimd.collective_compute(
                        "AllGather", OP.bypass, replica_groups=rg,
                        ins=[agin3[:, :]], outs=[table3[:, :]])

                agg3 = aggregate(t3_view, 16, f32, 16, tokds[2], shard3)

                # =============== log_softmax tail ===============
                def _bcast16(t):
                    ap = t[:, :]
                    return dataclasses.replace(ap, ap=type(ap.ap)(
                        [[NB, P], [1, NB], [0, 16]]))

                u = shard3  # reuse [P, NB*16]
                nc.vector.tensor_tensor(
                    out=u[:].rearrange("p (b f) -> p b f", b=NB),
                    in0=agg3[:, :NB * 16].rearrange(
                        "p (b f) -> p b f", b=NB),
                    in1=_bcast_blk(dinv_t, 0, NB, 16), op=OP.mult)
                nc.vector.tensor_tensor(
                    out=u[:].rearrange("p (b f) -> p b f", b=NB),
                    in0=u[:].rearrange("p (b f) -> p b f", b=NB),
                    in1=_bcast_row16(b3t_t, NB), op=OP.add)
                nc.vector.tensor_reduce(out=mx[:],
                                        in_=u[:].rearrange(
                                            "p (b f) -> p b f", b=NB),
                                        axis=mybir.AxisListType.X, op=OP.max)
                u3 = u[:].rearrange("p (b f) -> p b f", f=16)
                nc.vector.tensor_tensor(out=u3, in0=u3, in1=_bcast16(mx),
                                        op=OP.subtract)
                et = x1  # reuse first NB*16 cols as exp scratch
                nc.scalar.activation(out=et[:, :NB * 16], in_=u[:],
                                     func=AF.Exp)
                nc.vector.tensor_reduce(out=sm[:],
                                        in_=et[:, :NB * 16].rearrange(
                                            "p (b f) -> p b f", b=NB),
                                        axis=mybir.AxisListType.X, op=OP.add)
                nc.scalar.activation(out=lg[:], in_=sm[:], func=AF.Ln)
                nc.vector.tensor_tensor(out=u3, in0=u3, in1=_bcast16(lg),
                                        op=OP.subtract)
                nc.sync.dma_start(
                    out=out_ext[:, :].rearrange("(p b) f -> p b f", p=P),
                    in_=u[:].rearrange("p (b f) -> p b f", b=NB))

    nc.finalize()

    # Tile assigns SWDGE completion-sem lanes (DMASW0..7) round-robin in its
    # own scheduled order, and the runtime locks each lane to the first queue
    # that uses it. Make queue assignment consistent by construction:
    # queue_num := assigned lane % 4 for every SWDGE gather/scatter.
    from concourse.tile_scheduler import PROC_NAME_TO_IDX
    sw_lane_base = PROC_NAME_TO_IDX["DMASW0"]
    for inst in nc.inst_map.values():
        if isinstance(inst, (mybir.InstDMAGatherAnt,
                             mybir.InstDMAScatterAddAnt)):
            proc = getattr(inst, "bass_scheduled_proc", None)
            if proc is not None and sw_lane_base <= proc < sw_lane_base + 8:
                inst.queue_num = (proc - sw_lane_base) % 4
    return nc


# --------------------------------------------------------------------------

def _run_pjrt_bench(nc, in_maps, n_cores, iters):
    """Mirror bass2jax.run_bass_via_pjrt but keep the jitted callable and
    wall-clock `iters` repeat executions with device-resident inputs.
    Returns (results, times_s)."""
    import time

    import jax
    from jax.sharding import Mesh, NamedSharding, PartitionSpec
    from jax.experimental.shard_map import shard_map

    from concourse import bass2jax, mybir

    bass2jax.install_neuronx_cc_hook()
    assert nc.dbg_addr is None

    partition_name = (nc.partition_id_tensor.name
                      if nc.partition_id_tensor else None)
    in_names, out_names, out_avals, zero_outs = [], [], [], []
    for alloc in nc.m.functions[0].allocations:
        if not isinstance(alloc, mybir.MemoryLocationSet):
            continue
        name = alloc.memorylocations[0].name
        if alloc.kind == "ExternalInput":
            if name != partition_name:
                in_names.append(name)
        elif alloc.kind == "ExternalOutput":
            shape = tuple(alloc.tensor_shape)
            dtype = mybir.dt.np(alloc.dtype)
            out_names.append(name)
            out_avals.append(jax.core.ShapedArray(shape, dtype))
            zero_outs.append(np.zeros(shape, dtype))
    n_params = len(in_names)
    n_outs = len(out_avals)
    all_in_names = list(in_names) + out_names
    if partition_name is not None:
        all_in_names.append(partition_name)
    donate = tuple(range(n_params, n_params + n_outs))

    def _body(*args):
        operands = list(args)
        if partition_name is not None:
            operands.append(bass2jax.partition_id_tensor())
        outs = bass2jax._bass_exec_p.bind(
            *operands,
            out_avals=tuple(out_avals),
            in_names=tuple(all_in_names),
            out_names=tuple(out_names),
            lowering_input_output_aliases=(),
            sim_require_finite=True,
            sim_require_nnan=True,
            nc=nc,
        )
        return tuple(outs)

    devices = jax.devices()[:n_cores]
    mesh = Mesh(np.asarray(devices), ("core",))
    in_specs = (PartitionSpec("core"),) * (n_params + n_outs)
    out_specs = (PartitionSpec("core"),) * n_outs
    sharded = jax.jit(
        shard_map(_body, mesh=mesh, in_specs=in_specs, out_specs=out_specs,
                  check_rep=False),
        donate_argnums=donate, keep_unused=True)

    concat_in = [
        np.concatenate([np.asarray(in_maps[c][nm]) for c in range(n_cores)],
                       axis=0)
        for nm in in_names
    ]
    sh = NamedSharding(mesh, PartitionSpec("core"))
    dev_in = [jax.device_put(a, sh) for a in concat_in]
    zshapes = [(n_cores * z.shape[0], *z.shape[1:]) for z in zero_outs]

    def zset():
        return [jax.device_put(np.zeros(s, z.dtype), sh)
                for s, z in zip(zshapes, zero_outs)]

    out_arrs = sharded(*dev_in, *zset())
    jax.block_until_ready(out_arrs)
    results = [
        {nm: np.asarray(out_arrs[i]).reshape(n_cores, *out_avals[i].shape)[c]
         for i, nm in enumerate(out_names)}
        for c in range(n_cores)
    ]

    times = []
    if iters > 0:
        zsets = [zset() for _ in range(iters)]
        jax.block_until_ready(zsets)
        for i in range(iters):
            t0 = time.perf_counter()
            o = sharded(*dev_in, *zsets[i])
            jax.block_until_ready(o)
            times.append(time.perf_counter() - t0)
    return results, times


def kernel(**inputs):
    x = np.asarray(inputs["x"], dtype=np.float32)
    edge_index = np.asarray(inputs["edge_index"])
    W1 = np.asarray(inputs["W1"], dtype=np.float32)
    b1 = np.asarray(inputs["b1"], dtype=np.float32)
    W2 = np.asarray(inputs["W2"], dtype=np.float32)
    b2 = np.asarray(inputs["b2"], dtype=np.float32)
    W3 = np.asarray(inputs["W3"], dtype=np.float32)
    b3 = np.asarray(inputs["b3"], dtype=np.float32)

    meta, perm, idx_cat, cidx_cat = _prep(edge_index)
    in_maps = _pack_inputs(x, W1, b1, W2, b2, W3, b3, perm, idx_cat,
                           cidx_cat)
    nc = _build(meta)

    bench_iters = int(os.environ.get("GCN_BENCH_ITERS", "0"))
    if bench_iters > 0:
        results, times = _run_pjrt_bench(nc, in_maps, N_CORES, bench_iters)
        kernel.last_times = times
    else:
        from concourse.bass_utils import run_bass_kernel_spmd
        res = run_bass_kernel_spmd(nc, in_maps,
                                   core_ids=list(range(N_CORES)))
        results = res.results
        kernel.last_times = []

    out = np.empty((N_NODES, 10), dtype=np.float32)
    core_of, r_loc = perm["core_of"], perm["r_loc"]
    for c in range(N_CORES):
        oc = results[c]["out"]
        ids = np.flatnonzero(core_of == c)
        out[ids] = oc[r_loc[ids], :10]
    return out
